# revision 33
# baseline (speedup 1.0000x reference)
"""Trainium2 Bass kernel: 4D-CNN ResNet Bottleneck block, SPMD over 8 NeuronCores.

Problem (hardcoded): x[2,256,8,16,16,16] ->
  relu(bn3(conv1x1_256(relu(bn2(conv3x3x3x3(relu(bn1(conv1x1_64(x)))))))) + x)
BatchNorms use training-mode batch stats over (B,T,D,H,W).

Sharding: 16 (b,t) slices -> 2 owned t-slices/core; each core's input slab
includes the +-1 t halo (zero padded at boundaries), so no activation
exchange is needed. Three tiny AllReduces merge the BN statistics.

conv2 structure (cost-model-shaped): the 3x3x3x3 kernel is decomposed into
27 spatial taps x 4 input slab slices, with BOTH packing tricks:
  - M-packing: one input slice feeds both owned output t-slices with
    different kt weights, so M=128 = 2 t-outs x 64 channels.
  - K-packing: y1's upper partition half stores a copy shifted by one
    w-column, so a single K=128 access pattern delivers taps kw and kw+1
    simultaneously (kw in {0,1} paired; kw=2 runs as K=64 singles, read
    from the shifted copy at window kw=1 when the weights live on the
    upper partitions).
This cuts conv2 from 1296 to 576 matmuls of [*,*,512]; the TensorE cost is
proportional to output columns only.

DMAs are batched into large transfers (the per-instruction HWDGE setup is
expensive and serialized): x streams in 18 chunks, the residual in 4, the
output in 16.

Precision: conv1/conv2 bf16, conv3 float32r, residual injected into conv3
PSUM via an fp16 diag(1/bn3_scale) matmul so the epilogue is one ScalarE
pass. BN accumulators and conv accumulation stay fp32.
"""

import functools
import os

import numpy as np

# ---- problem constants --------------------------------------------------
B, C, T, D, H, W = 2, 256, 8, 16, 16, 16
PL = 64            # bottleneck planes
O3 = 4 * PL        # final channels (256)
NCORES = 8
EPS = 1e-5

TPC = 2                    # owned t-slices per core
SLAB = TPC + 2             # slab slices incl halo
DHW = D * H * W            # 4096
NSP_OWN = TPC * (D // 2)   # 16 owned superplanes (d-pairs)
NPOS_OWN = TPC * DHW       # 8192 positions per core
PW, PH, PD = W + 2, H + 2, D + 2
PSL = PD * PH * PW         # 5832 padded elems per slice
Y1COLS = 1 + SLAB * PSL + 1

# packed conv2 weight tile [128, 5184] column offsets. The s'={1,2} blocks
# come first so they can ship as one early DMA (conv2's first matmuls).
W2_PAIR1 = 0        # s'=1 pairs: 9 x M128
W2_PAIR2 = 1152     # s'=2 pairs: 9 x M128
W2_SING12 = 2304    # kw=2 singles for s'=1 (rows 0:64) / s'=2 (rows 64:128)
W2_PAIR0 = 3456     # s'=0 pairs: 9 x M64
W2_PAIR3 = 4032     # s'=3 pairs: 9 x M64
W2_SING03 = 4608    # kw=2 singles for s'=0 (rows 0:64) / s'=3 (rows 64:128)
W2_COLS = 5184
W2_SPLIT = 3456     # early DMA covers [0, W2_SPLIT)

MM_DT = os.environ.get("KERNEL_MM_DT", "float32r")  # conv3 matmul dtype
C2_DT = os.environ.get("KERNEL_C2_DT", "bfloat16")  # conv1/conv2 matmul dtype

LAST_RESULT = None  # BassKernelResults of the most recent run (for test.py)


@functools.lru_cache(maxsize=4)
def _build(mm_dt_name, c2_dt_name, collectives=True):
    from contextlib import ExitStack

    import concourse.mybir as mybir
    import concourse.tile as tile
    from concourse import bacc

    f32 = mybir.dt.float32
    mmdt = getattr(mybir.dt, mm_dt_name)
    c2dt = getattr(mybir.dt, c2_dt_name)
    fp16 = mybir.dt.float16
    AF = mybir.ActivationFunctionType
    AL = mybir.AluOpType

    nc = bacc.Bacc(
        "TRN2",
        target_bir_lowering=False,
        debug=False,
        enable_asserts=False,
        num_devices=NCORES,
    )

    xsb = nc.dram_tensor("xsb", [2, 128, SLAB * DHW], c2dt,
                         kind="ExternalInput").ap()
    xres = nc.dram_tensor("xres", [2, 128, NPOS_OWN], fp16,
                          kind="ExternalInput").ap()
    idm = nc.dram_tensor("idm", [128, 128], fp16, kind="ExternalInput").ap()
    w1t = nc.dram_tensor("w1t", [128, 4 * PL], c2dt, kind="ExternalInput").ap()
    w2t = nc.dram_tensor("w2t", [128, W2_COLS], c2dt, kind="ExternalInput").ap()
    w3t = nc.dram_tensor("w3t", [128, O3], f32, kind="ExternalInput").ap()
    gb1 = nc.dram_tensor("gb1", [128, 2], f32, kind="ExternalInput").ap()
    gb2 = nc.dram_tensor("gb2", [128, 2], f32, kind="ExternalInput").ap()
    gb3 = nc.dram_tensor("gb3", [128, 4], f32, kind="ExternalInput").ap()
    tmask = nc.dram_tensor("tmask", [128, SLAB], f32, kind="ExternalInput").ap()
    # bf16 output (host casts to f32): halves the store stream on the tail
    out = nc.dram_tensor("out", [2, 128, NPOS_OWN], c2dt,
                         kind="ExternalOutput").ap()

    cc1_in = nc.dram_tensor("cc1_in", [64, 2], f32).ap()
    cc1_out = nc.dram_tensor("cc1_out", [64, 2], f32, addr_space="Shared").ap()
    cc2_in = nc.dram_tensor("cc2_in", [128, 2], f32).ap()
    cc2_out = nc.dram_tensor("cc2_out", [128, 2], f32, addr_space="Shared").ap()
    cc3_in = nc.dram_tensor("cc3_in", [128, 4], f32).ap()
    cc3_out = nc.dram_tensor("cc3_out", [128, 4], f32, addr_space="Shared").ap()
    RG = [list(range(NCORES))]

    def allreduce(cin, cout):
        if collectives:
            nc.gpsimd.collective_compute(
                "AllReduce", AL.add, replica_groups=RG,
                ins=[cin], outs=[cout],
            )
        else:  # timing-sim variant: stand-in DMA with the same deps
            nc.sync.dma_start(out=cout, in_=cin)

    with tile.TileContext(nc) as tc, ExitStack() as st:
        const = st.enter_context(tc.tile_pool(name="const", bufs=1))
        smalls = st.enter_context(tc.tile_pool(name="smalls", bufs=1))

        def sm(shape, nm):
            return smalls.tile(shape, f32, tag=nm, name=nm)

        # ---- persistent SBUF tensors ---------------------------------
        w1sb = const.tile([128, 4 * PL], c2dt, tag="w1sb", name="w1sb")
        w2sb = const.tile([128, W2_COLS], c2dt, tag="w2sb", name="w2sb")
        w3sb = const.tile([128, O3], mmdt, tag="w3sb", name="w3sb")
        gb1sb = sm([128, 2], "gb1sb")
        gb2sb = sm([128, 2], "gb2sb")
        gb3sb = sm([128, 4], "gb3sb")
        tmsb = sm([128, SLAB], "tmsb")
        y2 = const.tile([128, NSP_OWN * 256], f32, tag="y2", name="y2")
        st1 = sm([64, NSP_OWN * 6], "st1")
        st2 = sm([128, 8 * 6], "st2")
        st3 = sm([128, 32 * 6], "st3")
        idmsb = const.tile([128, 128], fp16, tag="idmsb", name="idmsb")
        diag3 = const.tile([128, 256], fp16, tag="diag3", name="diag3")
        stg = st.enter_context(tc.tile_pool(name="stg", bufs=1))

        # ---- input DMA issue (SP queue; order = HWDGE order) ---------
        nc.sync.dma_start(out=w1sb[:], in_=w1t[:])

        # x slab chunks. s'=1's first d-half is split into [128,1024]
        # chunks so conv1's first matmul starts ~1.5us earlier.
        xin = st.enter_context(tc.tile_pool(name="xin", bufs=6))
        xin2 = st.enter_context(tc.tile_pool(name="xin2", bufs=8))
        xin0 = st.enter_context(tc.tile_pool(name="xin0", bufs=4))
        xchunk = {}  # (s', cb, quarter) -> (tile, base_dp, cols)

        def load_x(s, small_head):
            for cb in range(2):
                if small_head:
                    for q in range(2):
                        t = xin0.tile([128, 1024], c2dt, tag="xc1k")
                        nc.sync.dma_start(
                            out=t[:],
                            in_=xsb[cb, :, s * DHW + q * 1024:
                                    s * DHW + (q + 1) * 1024])
                        xchunk[(s, cb, q)] = (t, 2 * q, 1024)
                else:
                    t = xin.tile([128, 2048], c2dt, tag="xc2k")
                    nc.sync.dma_start(
                        out=t[:], in_=xsb[cb, :, s * DHW:s * DHW + 2048])
                    xchunk[(s, cb, 0)] = (t, 0, 2048)
                    xchunk[(s, cb, 1)] = (t, 0, 2048)
                t = xin.tile([128, 2048], c2dt, tag="xc2k")
                nc.sync.dma_start(
                    out=t[:], in_=xsb[cb, :, s * DHW + 2048:s * DHW + 4096])
                xchunk[(s, cb, 2)] = (t, 4, 2048)
                xchunk[(s, cb, 3)] = (t, 4, 2048)

        def xslice(s, cb, dp):
            tl, base_dp, cols = xchunk[(s, cb, dp // 2)]
            off = (dp - base_dp) * 512
            return tl[:, off:off + 512]

        load_x(1, True)
        load_x(2, False)
        # halo slices: d-half 0 of both before d-half 1 (halo compute and
        # BN1 apply run dp-major, so early dp need all four slices first)
        for cb in range(2):
            t = xin2.tile([128, 2048], c2dt, tag="xh2k")
            nc.sync.dma_start(out=t[:], in_=xsb[cb, :, 0:2048])
            xchunk[(0, cb, 0)] = (t, 0, 2048)
            xchunk[(0, cb, 1)] = (t, 0, 2048)
        for cb in range(2):
            t = xin2.tile([128, 2048], c2dt, tag="xh2k")
            nc.sync.dma_start(
                out=t[:], in_=xsb[cb, :, 3 * DHW:3 * DHW + 2048])
            xchunk[(3, cb, 0)] = (t, 0, 2048)
            xchunk[(3, cb, 1)] = (t, 0, 2048)
        # conv2's first weights ship right after the halo-h0 slices
        nc.sync.dma_start(out=w2sb[:, 0:W2_SPLIT], in_=w2t[:, 0:W2_SPLIT])
        nc.sync.dma_start(out=gb1sb[:], in_=gb1[:])
        nc.sync.dma_start(out=tmsb[:], in_=tmask[:])
        for s in (0, 3):
            for cb in range(2):
                t = xin2.tile([128, 2048], c2dt, tag="xh2k")
                nc.sync.dma_start(
                    out=t[:], in_=xsb[cb, :, s * DHW + 2048:s * DHW + 4096])
                xchunk[(s, cb, 2)] = (t, 4, 2048)
                xchunk[(s, cb, 3)] = (t, 4, 2048)
        nc.sync.dma_start(out=w2sb[:, W2_SPLIT:], in_=w2t[:, W2_SPLIT:])
        # w3 staged through f32 + rounding copy (float32r walrus rule)
        t = stg.tile([128, O3], f32, tag="stg")
        nc.sync.dma_start(out=t[:], in_=w3t[:])
        nc.vector.tensor_copy(out=w3sb[:], in_=t[:])
        nc.sync.dma_start(out=gb2sb[:], in_=gb2[:])
        nc.sync.dma_start(out=gb3sb[:], in_=gb3[:])
        nc.sync.dma_start(out=idmsb[:], in_=idm[:])

        # ---- BN finalize helpers -------------------------------------
        def bn_reduce_prep(mv, arin):
            """arin[:,0]=local mean, arin[:,1]=local E[x^2]."""
            t = sm([mv.shape[0], 1], f"bnprep_t_{arin.name}")
            nc.vector.tensor_tensor(out=t[:], in0=mv[:, 0:1], in1=mv[:, 0:1],
                                    op=AL.mult)
            nc.vector.tensor_tensor(out=arin[:, 1:2], in0=mv[:, 1:2], in1=t[:],
                                    op=AL.add)
            nc.vector.tensor_copy(out=arin[:, 0:1], in_=mv[:, 0:1])

        def bn_finalize(sums, inv_n, g_ap, b_ap, scale, bias, nm):
            """sums[:,0]=sum(mean_l), sums[:,1]=sum(e2_l) -> scale/bias [P,1]."""
            P = sums.shape[0]
            me = sm([P, 2], f"me_{nm}")
            tt = sm([P, 1], f"tt_{nm}")
            rstd = sm([P, 1], f"rstd_{nm}")
            nc.vector.tensor_scalar_mul(me[:], sums[:], inv_n)
            nc.vector.tensor_tensor(out=tt[:], in0=me[:, 0:1], in1=me[:, 0:1],
                                    op=AL.mult)
            nc.vector.tensor_tensor(out=tt[:], in0=me[:, 1:2], in1=tt[:],
                                    op=AL.subtract)
            nc.vector.tensor_scalar_add(tt[:], tt[:], EPS)
            nc.vector.reciprocal(out=tt[:], in_=tt[:])
            nc.scalar.activation(rstd[:], tt[:], AF.Sqrt)
            nc.vector.tensor_tensor(out=scale[:], in0=g_ap, in1=rstd[:],
                                    op=AL.mult)
            nc.vector.tensor_tensor(out=tt[:], in0=me[:, 0:1], in1=scale[:],
                                    op=AL.mult)
            nc.vector.tensor_tensor(out=bias[:], in0=b_ap, in1=tt[:],
                                    op=AL.subtract)

        scale1 = sm([128, 1], "scale1")
        bias1 = sm([128, 1], "bias1")
        scale2 = sm([128, 1], "scale2")
        bias2 = sm([128, 1], "bias2")
        scale3 = sm([128, 2], "scale3")
        bias3 = sm([128, 2], "bias3")
        scmat1 = sm([128, SLAB], "scmat1")
        bimat1 = sm([128, SLAB], "bimat1")

        with tc.tile_pool(name="y1p", bufs=1) as y1pool:
            y1 = y1pool.tile([128, Y1COLS], c2dt, tag="y1", name="y1")
            # zero in half-slice chunks, low-d halves first: the dp-major
            # BN1 apply touches the low-d half of every slice first, so
            # those chunks must clear the (serial) Pool engine early
            for h in range(2):
                for s in range(SLAB):
                    lo = 1 + s * PSL + h * (PSL // 2)
                    hi = lo + PSL // 2
                    if h == 0 and s == 0:
                        lo = 0
                    if h == 1 and s == SLAB - 1:
                        hi = Y1COLS
                    nc.gpsimd.memset(y1[:, lo:hi], 0.0)

            y1lo5 = y1[0:64, 1:1 + SLAB * PSL].rearrange(
                "p (s d h w) -> p s d h w", s=SLAB, d=PD, h=PH, w=PW)
            y1all5 = y1[:, 1:1 + SLAB * PSL].rearrange(
                "p (s d h w) -> p s d h w", s=SLAB, d=PD, h=PH, w=PW)
            y1up5 = y1[64:128, 1:1 + SLAB * PSL].rearrange(
                "p (s d h w) -> p s d h w", s=SLAB, d=PD, h=PH, w=PW)

            # ======== conv1 (single pass) =============================
            # Owned slices (stats) first so the BN1 AllReduce fires early;
            # halo slices + staging overlap its latency. psum halves are
            # identical (w1's M-dup), giving free sources for the two
            # (lower / shifted-upper) y1 writes later.
            y1raw = y1pool.tile([128, 2 * NSP_OWN * 512], c2dt,
                                tag="y1raw", name="y1raw")
            s_order = (1, 2, 0, 3)
            uidx = {(s, dp): i * 8 + dp for i, s in enumerate(s_order)
                    for dp in range(8)}
            # owned first (stats -> AR1 fires early); halo dp-major so the
            # dp-major BN1 apply can start on dp=0 as soon as possible
            items = ([(1, dp) for dp in range(8)] + [(2, dp) for dp in range(8)]
                     + [(s, dp) for dp in range(4) for s in (0, 3)]
                     + [(s, dp) for dp in range(4, 8) for s in (0, 3)])
            with tc.tile_pool(name="ps2", bufs=8, space="PSUM") as ps2:
                for s, dp in items:
                    owned = s in (1, 2)
                    if True:
                        ps = ps2.tile([128, 512], f32, tag="c1p2")
                        for cb in range(2):
                            nc.tensor.matmul(
                                ps[:],
                                lhsT=w1sb[:, cb * 2 * PL:(cb + 1) * 2 * PL],
                                rhs=xslice(s, cb, dp),
                                start=(cb == 0), stop=(cb == 1),
                            )
                        u = uidx[(s, dp)]
                        if owned:
                            sp = (s - 1) * 8 + dp
                            nc.vector.bn_stats(
                                out=st1[:, sp * 6:(sp + 1) * 6],
                                in_=ps[0:64, :])
                        # drain psum: stats on DVE, all copies on ACT (a
                        # halo copy on DVE would park in front of bn_aggr —
                        # the AR1 gate — and stall it on halo-x arrival)
                        nc.scalar.copy(
                            out=y1raw[:, u * 512:(u + 1) * 512], in_=ps[:])
                        if s == 2 and dp == 7:
                            # owned stats complete: start the AR1 chain now.
                            # Its small DMAs ride the ACT queue (the SP
                            # queue is head-of-line blocked by halo-x
                            # ring-buffer waits) and high_priority jumps
                            # them ahead of the staging-copy backlog.
                            with tc.high_priority():
                                mv1 = sm([64, 2], "mv1")
                                arin1 = sm([64, 2], "arin1")
                                nc.vector.bn_aggr(out=mv1[:], in_=st1[:])
                                bn_reduce_prep(mv1, arin1)
                                nc.sync.dma_start(out=cc1_in[:],
                                                  in_=arin1[:])
                                allreduce(cc1_in[:], cc1_out[:])
                                g1s = sm([128, 2], "g1s")
                                nc.sync.dma_start(out=g1s[0:64, :],
                                                  in_=cc1_out[:])
                                nc.sync.dma_start(out=g1s[64:128, :],
                                                  in_=cc1_out[:])
                                bn_finalize(g1s, 1.0 / NCORES, gb1sb[:, 0:1],
                                            gb1sb[:, 1:2], scale1, bias1,
                                            "bn1")
                                # per-slice masked scale/bias (zero invalid)
                                nc.vector.tensor_scalar_mul(scmat1[:],
                                                            tmsb[:],
                                                            scale1[:])
                                nc.vector.tensor_scalar_mul(bimat1[:],
                                                            tmsb[:],
                                                            bias1[:])

            # ---- BN1 + relu into the padded y1 layout ----------------
            # dp-major so conv2's dp=0 can start as early as possible.
            # Lower half: unshifted; upper half: shifted one w-column left
            # (upper[w] = y1[w+1]) to enable conv2's kw-pair K-packing.
            # tile_wait_until keeps the scheduler from parking these
            # scale1-gated ops at the ACT queue head in front of the
            # (much earlier ready) psum staging copies.
            st.enter_context(tc.tile_wait_until(1))
            # merged over dp-pairs: y1raw blocks for (s, 2q) and (s, 2q+1)
            # are adjacent, and so are their d-windows in y1
            for q in range(4):
                dlo = 4 * q + 1
                for s in range(SLAB):
                    u = uidx[(s, 2 * q)]
                    rlo = y1raw[0:64, u * 512:(u + 2) * 512].rearrange(
                        "p (d h w) -> p d h w", d=4, h=16, w=16)
                    rhi = y1raw[64:128, u * 512:(u + 2) * 512].rearrange(
                        "p (d h w) -> p d h w", d=4, h=16, w=16)
                    nc.scalar.activation(
                        y1lo5[:, s, dlo:dlo + 4, 1:17, 1:17], rlo, AF.Relu,
                        bias=bimat1[0:64, s:s + 1], scale=scmat1[0:64, s:s + 1])
                    nc.scalar.activation(
                        y1up5[:, s, dlo:dlo + 4, 1:17, 0:16], rhi, AF.Relu,
                        bias=bimat1[64:128, s:s + 1],
                        scale=scmat1[64:128, s:s + 1])

            # ======== conv2: 27 spatial taps x 4 slices, packed =======
            # Per d-pair dp one PSUM bank accumulates both owned t-outs:
            # psum[0:64] = t-out s=1, psum[64:128] = t-out s=2.
            def win(view, s, dp, kd, kh, kw):
                d0 = 2 * dp + kd
                return view[:, s, d0:d0 + 2, kh:kh + 16, kw:kw + 16]

            with tc.tile_pool(name="ps3", bufs=4, space="PSUM") as ps3:
                for dp in range(8):
                    ps = ps3.tile([128, 512], f32, tag="c2p")
                    first = [True]

                    def mm(outap, lhsT, rhs, stop=False):
                        nc.tensor.matmul(outap, lhsT=lhsT, rhs=rhs,
                                         start=first[0], stop=stop)
                        first[0] = False

                    for kd in range(3):
                        for kh in range(3):
                            b = (kd * 3 + kh)
                            # s'=1: M=128 pairs (kw 0+1) and kw=2 singles
                            mm(ps[:], w2sb[:, W2_PAIR1 + b * 128:
                                           W2_PAIR1 + (b + 1) * 128],
                               win(y1all5, 1, dp, kd, kh, 0))
                            mm(ps[:], w2sb[0:64, W2_SING12 + b * 128:
                                           W2_SING12 + (b + 1) * 128],
                               win(y1lo5, 1, dp, kd, kh, 2))
                            # s'=2: M=128
                            mm(ps[:], w2sb[:, W2_PAIR2 + b * 128:
                                           W2_PAIR2 + (b + 1) * 128],
                               win(y1all5, 2, dp, kd, kh, 0))
                            mm(ps[:], w2sb[64:128, W2_SING12 + b * 128:
                                           W2_SING12 + (b + 1) * 128],
                               win(y1up5, 2, dp, kd, kh, 1))
                    for kd in range(3):
                        for kh in range(3):
                            b = (kd * 3 + kh)
                            last = (kd == 2 and kh == 2)
                            # s'=0 -> only t-out s=1 (psum[0:64])
                            mm(ps[0:64, :], w2sb[:, W2_PAIR0 + b * 64:
                                                 W2_PAIR0 + (b + 1) * 64],
                               win(y1all5, 0, dp, kd, kh, 0))
                            mm(ps[0:64, :], w2sb[0:64, W2_SING03 + b * 64:
                                                 W2_SING03 + (b + 1) * 64],
                               win(y1lo5, 0, dp, kd, kh, 2), stop=last)
                    for kd in range(3):
                        for kh in range(3):
                            b = (kd * 3 + kh)
                            last = (kd == 2 and kh == 2)
                            # s'=3 -> only t-out s=2 (psum[64:128])
                            mm(ps[64:128, :], w2sb[:, W2_PAIR3 + b * 64:
                                                   W2_PAIR3 + (b + 1) * 64],
                               win(y1all5, 3, dp, kd, kh, 0))
                            mm(ps[64:128, :], w2sb[64:128, W2_SING03 + b * 64:
                                                   W2_SING03 + (b + 1) * 64],
                               win(y1up5, 3, dp, kd, kh, 1), stop=last)

                    nc.scalar.copy(out=y2[:, dp * 512:(dp + 1) * 512],
                                   in_=ps[:])
                    nc.vector.bn_stats(out=st2[:, dp * 6:(dp + 1) * 6],
                                       in_=y2[:, dp * 512:(dp + 1) * 512])

        # y1 freed here
        with tc.high_priority():
            mv2 = sm([128, 2], "mv2")
            arin2 = sm([128, 2], "arin2")
            nc.vector.bn_aggr(out=mv2[:], in_=st2[:])
            bn_reduce_prep(mv2, arin2)
            nc.sync.dma_start(out=cc2_in[:], in_=arin2[:])
            allreduce(cc2_in[:], cc2_out[:])
            fa = sm([128, 2], "fa2")
            fb = sm([128, 2], "fb2")
            nc.sync.dma_start(out=fa[:], in_=cc2_out[:])
            nc.sync.dma_start(out=fb[0:64, :], in_=cc2_out[64:128, :])
            nc.sync.dma_start(out=fb[64:128, :], in_=cc2_out[0:64, :])
            nc.vector.tensor_tensor(out=fa[:], in0=fa[:], in1=fb[:],
                                    op=AL.add)
            bn_finalize(fa, 1.0 / (2 * NCORES), gb2sb[:, 0:1], gb2sb[:, 1:2],
                        scale2, bias2, "bn2")

        # residual x tiles: 4 x [128, 4096] fp16; tile j = oh*2 + h covers
        # positions [h*4096, (h+1)*4096) of output-channel half oh. The
        # pool opens after y1 is freed, reusing its SBUF zone.
        x9p = st.enter_context(tc.tile_pool(name="x9p", bufs=4))
        xt9 = [x9p.tile([128, 4096], fp16, tag="xt9", name=f"xt9_{j}")
               for j in range(4)]
        # residual prefetch on the (idle by now) Pool queue: the dummy
        # scale2-dependent write pins these DMAs to the DMA-idle BN2
        # window, so their transfers can't steal bandwidth from the conv1
        # x stream (the tile scheduler is readiness-driven)
        for j in range(4):
            nc.vector.tensor_copy(out=xt9[j][:, 0:1], in_=scale2[:])
            nc.gpsimd.dma_start(
                out=xt9[j][:],
                in_=xres[j // 2, :, (j % 2) * 4096:(j % 2 + 1) * 4096])

        with tc.tile_pool(name="zp", bufs=1) as zpool:
            y2n = zpool.tile([128, NSP_OWN * 256], mmdt, tag="y2n", name="y2n")

            def c3_mm(ps4, dp, half, oh, stop=True):
                rhs = y2n[half * 64:(half + 1) * 64,
                          dp * 512:(dp + 1) * 512].bitcast(mmdt)
                pg = ps4.tile([128, 512], f32, tag="c3")
                nc.tensor.matmul(
                    pg[:],
                    lhsT=w3sb[half * 64:(half + 1) * 64,
                              oh * 128:(oh + 1) * 128].bitcast(mmdt),
                    rhs=rhs, start=True, stop=stop)
                return pg

            # ==== BN2 apply + conv3 pass 1 (stats), interleaved =======
            with tc.tile_pool(name="ps4", bufs=8, space="PSUM") as ps4:
                for dp in range(8):
                    nc.scalar.activation(
                        y2n[:, dp * 512:(dp + 1) * 512],
                        y2[:, dp * 512:(dp + 1) * 512],
                        AF.Relu, bias=bias2[:], scale=scale2[:])
                    for half in range(2):
                        sp = half * 8 + dp
                        for oh in range(2):
                            pg = c3_mm(ps4, dp, half, oh)
                            nc.vector.bn_stats(
                                out=st3[:, (oh * 16 + sp) * 6:
                                        (oh * 16 + sp) * 6 + 6],
                                in_=pg[:])

            # BN3 stats merge, vectorized [128,2]-wide over both oh halves:
            # arin3 cols = (mean0, mean1, e2_0, e2_1)
            with tc.high_priority():
                mv3a = sm([128, 2], "mv3a")
                mv3b = sm([128, 2], "mv3b")
                arin3 = sm([128, 4], "arin3")
                sq3 = sm([128, 2], "sq3")
                nc.vector.bn_aggr(out=mv3a[:], in_=st3[:, 0:96])
                nc.vector.bn_aggr(out=mv3b[:], in_=st3[:, 96:192])
                nc.vector.tensor_copy(out=arin3[:, 0:1], in_=mv3a[:, 0:1])
                nc.vector.tensor_copy(out=arin3[:, 1:2], in_=mv3b[:, 0:1])
                nc.vector.tensor_copy(out=arin3[:, 2:3], in_=mv3a[:, 1:2])
                nc.vector.tensor_copy(out=arin3[:, 3:4], in_=mv3b[:, 1:2])
                nc.vector.tensor_tensor(out=sq3[:], in0=arin3[:, 0:2],
                                        in1=arin3[:, 0:2], op=AL.mult)
                nc.vector.tensor_tensor(out=arin3[:, 2:4], in0=arin3[:, 2:4],
                                        in1=sq3[:], op=AL.add)
                nc.sync.dma_start(out=cc3_in[:], in_=arin3[:])
                allreduce(cc3_in[:], cc3_out[:])
                g3s = sm([128, 4], "g3s")
                nc.sync.dma_start(out=g3s[:], in_=cc3_out[:])
                recip3 = sm([128, 2], "recip3")
                me3 = sm([128, 4], "me3")
                v3 = sm([128, 2], "v3")
                rstd3 = sm([128, 2], "rstd3")
                nc.vector.tensor_scalar_mul(me3[:], g3s[:], 1.0 / NCORES)
                nc.vector.tensor_tensor(out=sq3[:], in0=me3[:, 0:2],
                                        in1=me3[:, 0:2], op=AL.mult)
                nc.vector.tensor_tensor(out=v3[:], in0=me3[:, 2:4],
                                        in1=sq3[:], op=AL.subtract)
                nc.vector.tensor_scalar_add(v3[:], v3[:], EPS)
                nc.vector.reciprocal(out=v3[:], in_=v3[:])
                nc.scalar.activation(rstd3[:], v3[:], AF.Sqrt)
                nc.vector.tensor_tensor(out=scale3[:], in0=gb3sb[:, 0:2],
                                        in1=rstd3[:], op=AL.mult)
                nc.vector.tensor_tensor(out=sq3[:], in0=me3[:, 0:2],
                                        in1=scale3[:], op=AL.mult)
                nc.vector.tensor_tensor(out=bias3[:], in0=gb3sb[:, 2:4],
                                        in1=sq3[:], op=AL.subtract)
                nc.vector.reciprocal(out=recip3[:], in_=scale3[:])
                for oh in range(2):
                    # diag(1/scale3): PE injects the residual into PSUM
                    nc.vector.tensor_scalar_mul(
                        diag3[:, oh * 128:(oh + 1) * 128], idmsb[:],
                        recip3[:, oh:oh + 1])

            # ==== conv3 pass 2 + fused BN3/residual/relu/store ========
            # Stores batched as [128,1024] per (oh, half, dp-pair).
            with tc.tile_pool(name="ps5", bufs=8, space="PSUM") as ps5, \
                 tc.tile_pool(name="fino", bufs=8) as fino:
                ftiles = {}
                for dp in range(8):
                    for half in range(2):
                        sp = half * 8 + dp
                        for oh in range(2):
                            pg = c3_mm(ps5, dp, half, oh, stop=False)
                            nc.tensor.matmul(
                                pg[:], lhsT=diag3[:, oh * 128:(oh + 1) * 128],
                                rhs=xt9[oh * 2 + half][:, dp * 512:
                                                       (dp + 1) * 512],
                                start=False, stop=True)
                            if dp % 2 == 0:
                                ftiles[(oh, half)] = fino.tile(
                                    [128, 1024], c2dt, tag="o9",
                                    name=f"o9_{dp}_{half}_{oh}")
                            ft = ftiles[(oh, half)]
                            nc.scalar.activation(
                                ft[:, (dp % 2) * 512:(dp % 2) * 512 + 512],
                                pg[:], AF.Relu,
                                bias=bias3[:, oh:oh + 1],
                                scale=scale3[:, oh:oh + 1])
                            if dp % 2 == 1:
                                nc.sync.dma_start(
                                    out=out[oh, :, sp * 512 - 512:
                                            sp * 512 + 512],
                                    in_=ft[:])

    nc.compile()
    return nc


# ---- host-side input prep / output assembly -----------------------------

def _prep_w2(w2):
    """Pack w2 [64,64,3,3,3,3] into the [128, W2_COLS] lhsT tile layout.

    kt for (t-out column half m-half, input slice s') is kt = s' - s_out + 1
    with s_out = 1 + m_half. Pair blocks: rows 0:64 = tap kw (=0), rows
    64:128 = tap kw+1 (=1, via the shifted y1 copy). Singles: tap kw=2.
    """
    f4 = np.float32
    w = np.ascontiguousarray(w2, f4)  # [o, c, kt, kd, kh, kw]
    res = np.zeros((128, W2_COLS), f4)
    for kd in range(3):
        for kh in range(3):
            b = kd * 3 + kh
            # pairs for s'=1, s'=2 (M=128)
            for base, sp in ((W2_PAIR1, 1), (W2_PAIR2, 2)):
                for mh in range(2):     # m-half: t-out s_out = 1 + mh
                    kt = sp - (1 + mh) + 1
                    for kw in range(2):
                        res[kw * 64:kw * 64 + 64,
                            base + b * 128 + mh * 64:
                            base + b * 128 + mh * 64 + 64] = \
                            w[:, :, kt, kd, kh, kw].T
            # pairs for s'=0 (t-out 1, kt=0) / s'=3 (t-out 2, kt=2)
            for base, kt in ((W2_PAIR0, 0), (W2_PAIR3, 2)):
                for kw in range(2):
                    res[kw * 64:kw * 64 + 64,
                        base + b * 64:base + b * 64 + 64] = \
                        w[:, :, kt, kd, kh, kw].T
            # kw=2 singles: s'=1 rows 0:64, s'=2 rows 64:128 (M=128)
            for rh, sp in ((0, 1), (1, 2)):
                for mh in range(2):
                    kt = sp - (1 + mh) + 1
                    res[rh * 64:rh * 64 + 64,
                        W2_SING12 + b * 128 + mh * 64:
                        W2_SING12 + b * 128 + mh * 64 + 64] = \
                        w[:, :, kt, kd, kh, 2].T
            # kw=2 singles: s'=0 rows 0:64 (kt=0), s'=3 rows 64:128 (kt=2)
            for rh, kt in ((0, 0), (1, 2)):
                res[rh * 64:rh * 64 + 64,
                    W2_SING03 + b * 64:W2_SING03 + b * 64 + 64] = \
                    w[:, :, kt, kd, kh, 2].T
    return res


def _prep_inputs(x, w1, g1, b1, w2, g2, b2, w3, g3, b3):
    import ml_dtypes
    f4 = np.float32
    bf = ml_dtypes.bfloat16
    xr = np.ascontiguousarray(x, f4).reshape(B, C, T, DHW)

    w2t = _prep_w2(w2).astype(bf)

    w1T = np.ascontiguousarray(w1, f4).T.reshape(2, 128, PL)  # [cb, k, o]
    w1t = np.ascontiguousarray(
        np.concatenate([np.concatenate([w1T[cb]] * 2, 1) for cb in range(2)], 1)
    ).astype(bf)
    w3t = np.concatenate([np.ascontiguousarray(w3, f4).T] * 2, 0).copy()

    gb1 = np.stack([np.asarray(g1, f4), np.asarray(b1, f4)], 1)
    gb1 = np.concatenate([gb1, gb1], 0)
    gb2 = np.stack([np.asarray(g2, f4), np.asarray(b2, f4)], 1)
    gb2 = np.concatenate([gb2, gb2], 0)
    g3r = np.asarray(g3, f4).reshape(2, 128).T
    b3r = np.asarray(b3, f4).reshape(2, 128).T
    gb3 = np.concatenate([g3r, b3r], 1).copy()  # [128,4]

    in_maps = []
    for core in range(NCORES):
        b = core // 4
        t0 = 2 * (core % 4)
        xslab = np.zeros((C, SLAB, DHW), f4)
        tm = np.zeros((SLAB,), f4)
        for si, gt in enumerate(range(t0 - 1, t0 + 3)):
            if 0 <= gt < T:
                xslab[:, si] = xr[b, :, gt]
                tm[si] = 1.0
        xs2 = xslab.reshape(2, 128, SLAB * DHW)
        in_maps.append({
            "xsb": np.ascontiguousarray(xs2).astype(bf),
            "xres": np.ascontiguousarray(
                xs2[:, :, DHW:DHW + NPOS_OWN]).astype(np.float16),
            "idm": np.eye(128, dtype=np.float16),
            "w1t": w1t, "w2t": w2t, "w3t": w3t,
            "gb1": gb1, "gb2": gb2, "gb3": gb3,
            "tmask": np.broadcast_to(tm, (128, SLAB)).copy(),
        })
    return in_maps


def kernel(x, w1, g1, b1, w2, g2, b2, w3, g3, b3):
    global LAST_RESULT
    from concourse.bass_utils import run_bass_kernel_spmd

    nc = _build(MM_DT, C2_DT)
    in_maps = _prep_inputs(x, w1, g1, b1, w2, g2, b2, w3, g3, b3)
    res = run_bass_kernel_spmd(nc, in_maps, core_ids=list(range(NCORES)))
    LAST_RESULT = res

    full = np.empty((B, C, T, D, H, W), np.float32)
    for core in range(NCORES):
        b = core // 4
        t0 = 2 * (core % 4)
        o = np.asarray(res.results[core]["out"], np.float32)
        full[b, :, t0:t0 + TPC] = o.reshape(C, TPC, D, H, W)
    return full


# revision 34
# speedup vs baseline: 1.0187x; 1.0187x over previous
"""Trainium2 Bass kernel: 4D-CNN ResNet Bottleneck block, SPMD over 8 NeuronCores.

Problem (hardcoded): x[2,256,8,16,16,16] ->
  relu(bn3(conv1x1_256(relu(bn2(conv3x3x3x3(relu(bn1(conv1x1_64(x)))))))) + x)
BatchNorms use training-mode batch stats over (B,T,D,H,W).

Sharding: 16 (b,t) slices -> 2 owned t-slices/core; each core's input slab
includes the +-1 t halo (zero padded at boundaries), so no activation
exchange is needed. Three tiny AllReduces merge the BN statistics.

conv2 structure (cost-model-shaped): the 3x3x3x3 kernel is decomposed into
27 spatial taps x 4 input slab slices, with BOTH packing tricks:
  - M-packing: one input slice feeds both owned output t-slices with
    different kt weights, so M=128 = 2 t-outs x 64 channels.
  - K-packing: y1's upper partition half stores a copy shifted by one
    w-column, so a single K=128 access pattern delivers taps kw and kw+1
    simultaneously (kw in {0,1} paired; kw=2 runs as K=64 singles, read
    from the shifted copy at window kw=1 when the weights live on the
    upper partitions).
This cuts conv2 from 1296 to 576 matmuls of [*,*,512]; the TensorE cost is
proportional to output columns only.

DMAs are batched into large transfers (the per-instruction HWDGE setup is
expensive and serialized): x streams in 18 chunks, the residual in 4, the
output in 16.

Precision: conv1/conv2 bf16, conv3 float32r, residual injected into conv3
PSUM via an fp16 diag(1/bn3_scale) matmul so the epilogue is one ScalarE
pass. BN accumulators and conv accumulation stay fp32.
"""

import functools
import os

import numpy as np

# ---- problem constants --------------------------------------------------
B, C, T, D, H, W = 2, 256, 8, 16, 16, 16
PL = 64            # bottleneck planes
O3 = 4 * PL        # final channels (256)
NCORES = 8
EPS = 1e-5

TPC = 2                    # owned t-slices per core
SLAB = TPC + 2             # slab slices incl halo
DHW = D * H * W            # 4096
NSP_OWN = TPC * (D // 2)   # 16 owned superplanes (d-pairs)
NPOS_OWN = TPC * DHW       # 8192 positions per core
PW, PH, PD = W + 2, H + 2, D + 2
PSL = PD * PH * PW         # 5832 padded elems per slice
Y1COLS = 1 + SLAB * PSL + 1

# packed conv2 weight tile [128, 5184] column offsets. The s'={1,2} blocks
# come first so they can ship as one early DMA (conv2's first matmuls).
W2_PAIR1 = 0        # s'=1 pairs: 9 x M128
W2_PAIR2 = 1152     # s'=2 pairs: 9 x M128
W2_SING12 = 2304    # kw=2 singles for s'=1 (rows 0:64) / s'=2 (rows 64:128)
W2_PAIR0 = 3456     # s'=0 pairs: 9 x M64
W2_PAIR3 = 4032     # s'=3 pairs: 9 x M64
W2_SING03 = 4608    # kw=2 singles for s'=0 (rows 0:64) / s'=3 (rows 64:128)
W2_COLS = 5184
W2_SPLIT = 3456     # early DMA covers [0, W2_SPLIT)

MM_DT = os.environ.get("KERNEL_MM_DT", "float32r")  # conv3 matmul dtype
C2_DT = os.environ.get("KERNEL_C2_DT", "bfloat16")  # conv1/conv2 matmul dtype

LAST_RESULT = None  # BassKernelResults of the most recent run (for test.py)


@functools.lru_cache(maxsize=4)
def _build(mm_dt_name, c2_dt_name, collectives=True):
    from contextlib import ExitStack

    import concourse.mybir as mybir
    import concourse.tile as tile
    from concourse import bacc

    f32 = mybir.dt.float32
    mmdt = getattr(mybir.dt, mm_dt_name)
    c2dt = getattr(mybir.dt, c2_dt_name)
    fp16 = mybir.dt.float16
    AF = mybir.ActivationFunctionType
    AL = mybir.AluOpType

    nc = bacc.Bacc(
        "TRN2",
        target_bir_lowering=False,
        debug=False,
        enable_asserts=False,
        num_devices=NCORES,
    )

    xsb = nc.dram_tensor("xsb", [2, 128, SLAB * DHW], c2dt,
                         kind="ExternalInput").ap()
    xres = nc.dram_tensor("xres", [2, 128, NPOS_OWN], fp16,
                          kind="ExternalInput").ap()
    idm = nc.dram_tensor("idm", [128, 128], fp16, kind="ExternalInput").ap()
    w1t = nc.dram_tensor("w1t", [128, 4 * PL], c2dt, kind="ExternalInput").ap()
    w2t = nc.dram_tensor("w2t", [128, W2_COLS], c2dt, kind="ExternalInput").ap()
    w3t = nc.dram_tensor("w3t", [128, O3], f32, kind="ExternalInput").ap()
    gb1 = nc.dram_tensor("gb1", [128, 2], f32, kind="ExternalInput").ap()
    gb2 = nc.dram_tensor("gb2", [128, 2], f32, kind="ExternalInput").ap()
    gb3 = nc.dram_tensor("gb3", [128, 4], f32, kind="ExternalInput").ap()
    tmask = nc.dram_tensor("tmask", [128, SLAB], f32, kind="ExternalInput").ap()
    # bf16 output (host casts to f32): halves the store stream on the tail
    out = nc.dram_tensor("out", [2, 128, NPOS_OWN], c2dt,
                         kind="ExternalOutput").ap()

    cc1_in = nc.dram_tensor("cc1_in", [64, 2], f32).ap()
    cc1_out = nc.dram_tensor("cc1_out", [64, 2], f32, addr_space="Shared").ap()
    cc2_in = nc.dram_tensor("cc2_in", [128, 2], f32).ap()
    cc2_out = nc.dram_tensor("cc2_out", [128, 2], f32, addr_space="Shared").ap()
    cc3_in = nc.dram_tensor("cc3_in", [128, 4], f32).ap()
    cc3_out = nc.dram_tensor("cc3_out", [128, 4], f32, addr_space="Shared").ap()
    RG = [list(range(NCORES))]

    def allreduce(cin, cout):
        if collectives:
            nc.gpsimd.collective_compute(
                "AllReduce", AL.add, replica_groups=RG,
                ins=[cin], outs=[cout],
            )
        else:  # timing-sim variant: stand-in DMA with the same deps
            nc.sync.dma_start(out=cout, in_=cin)

    with tile.TileContext(nc) as tc, ExitStack() as st:
        const = st.enter_context(tc.tile_pool(name="const", bufs=1))
        smalls = st.enter_context(tc.tile_pool(name="smalls", bufs=1))

        def sm(shape, nm):
            return smalls.tile(shape, f32, tag=nm, name=nm)

        # ---- persistent SBUF tensors ---------------------------------
        w1sb = const.tile([128, 4 * PL], c2dt, tag="w1sb", name="w1sb")
        w2sb = const.tile([128, W2_COLS], c2dt, tag="w2sb", name="w2sb")
        w3sb = const.tile([128, O3], mmdt, tag="w3sb", name="w3sb")
        gb1sb = sm([128, 2], "gb1sb")
        gb2sb = sm([128, 2], "gb2sb")
        gb3sb = sm([128, 4], "gb3sb")
        tmsb = sm([128, SLAB], "tmsb")
        y2 = const.tile([128, NSP_OWN * 256], f32, tag="y2", name="y2")
        st1 = sm([64, NSP_OWN * 6], "st1")
        st2 = sm([128, 8 * 6], "st2")
        st3 = sm([128, 32 * 6], "st3")
        idmsb = const.tile([128, 128], fp16, tag="idmsb", name="idmsb")
        diag3 = const.tile([128, 256], fp16, tag="diag3", name="diag3")
        stg = st.enter_context(tc.tile_pool(name="stg", bufs=1))

        # ---- input DMA issue (SP queue; order = HWDGE order) ---------
        nc.sync.dma_start(out=w1sb[:], in_=w1t[:])

        # x slab chunks. s'=1's first d-half is split into [128,1024]
        # chunks so conv1's first matmul starts ~1.5us earlier.
        xin = st.enter_context(tc.tile_pool(name="xin", bufs=6))
        xin2 = st.enter_context(tc.tile_pool(name="xin2", bufs=8))
        xin0 = st.enter_context(tc.tile_pool(name="xin0", bufs=4))
        xchunk = {}  # (s', cb, quarter) -> (tile, base_dp, cols)

        def load_x(s, small_head):
            for cb in range(2):
                if small_head:
                    for q in range(2):
                        t = xin0.tile([128, 1024], c2dt, tag="xc1k")
                        nc.sync.dma_start(
                            out=t[:],
                            in_=xsb[cb, :, s * DHW + q * 1024:
                                    s * DHW + (q + 1) * 1024])
                        xchunk[(s, cb, q)] = (t, 2 * q, 1024)
                else:
                    t = xin.tile([128, 2048], c2dt, tag="xc2k")
                    nc.sync.dma_start(
                        out=t[:], in_=xsb[cb, :, s * DHW:s * DHW + 2048])
                    xchunk[(s, cb, 0)] = (t, 0, 2048)
                    xchunk[(s, cb, 1)] = (t, 0, 2048)
                t = xin.tile([128, 2048], c2dt, tag="xc2k")
                nc.sync.dma_start(
                    out=t[:], in_=xsb[cb, :, s * DHW + 2048:s * DHW + 4096])
                xchunk[(s, cb, 2)] = (t, 4, 2048)
                xchunk[(s, cb, 3)] = (t, 4, 2048)

        def xslice(s, cb, dp):
            tl, base_dp, cols = xchunk[(s, cb, dp // 2)]
            off = (dp - base_dp) * 512
            return tl[:, off:off + 512]

        load_x(1, True)
        load_x(2, False)
        # halo slices: d-half 0 of both before d-half 1 (halo compute and
        # BN1 apply run dp-major, so early dp need all four slices first)
        for cb in range(2):
            t = xin2.tile([128, 2048], c2dt, tag="xh2k")
            nc.sync.dma_start(out=t[:], in_=xsb[cb, :, 0:2048])
            xchunk[(0, cb, 0)] = (t, 0, 2048)
            xchunk[(0, cb, 1)] = (t, 0, 2048)
        for cb in range(2):
            t = xin2.tile([128, 2048], c2dt, tag="xh2k")
            nc.sync.dma_start(
                out=t[:], in_=xsb[cb, :, 3 * DHW:3 * DHW + 2048])
            xchunk[(3, cb, 0)] = (t, 0, 2048)
            xchunk[(3, cb, 1)] = (t, 0, 2048)
        # conv2's first weights ship right after the halo-h0 slices
        nc.sync.dma_start(out=w2sb[:, 0:W2_SPLIT], in_=w2t[:, 0:W2_SPLIT])
        nc.sync.dma_start(out=gb1sb[:], in_=gb1[:])
        nc.sync.dma_start(out=tmsb[:], in_=tmask[:])
        for s in (0, 3):
            for cb in range(2):
                t = xin2.tile([128, 2048], c2dt, tag="xh2k")
                nc.sync.dma_start(
                    out=t[:], in_=xsb[cb, :, s * DHW + 2048:s * DHW + 4096])
                xchunk[(s, cb, 2)] = (t, 4, 2048)
                xchunk[(s, cb, 3)] = (t, 4, 2048)
        nc.sync.dma_start(out=w2sb[:, W2_SPLIT:], in_=w2t[:, W2_SPLIT:])
        # w3 staged through f32 + rounding copy (float32r walrus rule)
        t = stg.tile([128, O3], f32, tag="stg")
        nc.sync.dma_start(out=t[:], in_=w3t[:])
        nc.vector.tensor_copy(out=w3sb[:], in_=t[:])
        nc.sync.dma_start(out=gb2sb[:], in_=gb2[:])
        nc.sync.dma_start(out=gb3sb[:], in_=gb3[:])
        nc.sync.dma_start(out=idmsb[:], in_=idm[:])

        # ---- BN finalize helpers -------------------------------------
        def bn_reduce_prep(mv, arin):
            """arin[:,0]=local mean, arin[:,1]=local E[x^2]."""
            t = sm([mv.shape[0], 1], f"bnprep_t_{arin.name}")
            nc.vector.tensor_tensor(out=t[:], in0=mv[:, 0:1], in1=mv[:, 0:1],
                                    op=AL.mult)
            nc.vector.tensor_tensor(out=arin[:, 1:2], in0=mv[:, 1:2], in1=t[:],
                                    op=AL.add)
            nc.vector.tensor_copy(out=arin[:, 0:1], in_=mv[:, 0:1])

        def bn_finalize(sums, inv_n, g_ap, b_ap, scale, bias, nm):
            """sums[:,0]=sum(mean_l), sums[:,1]=sum(e2_l) -> scale/bias [P,1]."""
            P = sums.shape[0]
            me = sm([P, 2], f"me_{nm}")
            tt = sm([P, 1], f"tt_{nm}")
            rstd = sm([P, 1], f"rstd_{nm}")
            nc.vector.tensor_scalar_mul(me[:], sums[:], inv_n)
            nc.vector.tensor_tensor(out=tt[:], in0=me[:, 0:1], in1=me[:, 0:1],
                                    op=AL.mult)
            nc.vector.tensor_tensor(out=tt[:], in0=me[:, 1:2], in1=tt[:],
                                    op=AL.subtract)
            nc.vector.tensor_scalar_add(tt[:], tt[:], EPS)
            nc.vector.reciprocal(out=tt[:], in_=tt[:])
            nc.scalar.activation(rstd[:], tt[:], AF.Sqrt)
            nc.vector.tensor_tensor(out=scale[:], in0=g_ap, in1=rstd[:],
                                    op=AL.mult)
            nc.vector.tensor_tensor(out=tt[:], in0=me[:, 0:1], in1=scale[:],
                                    op=AL.mult)
            nc.vector.tensor_tensor(out=bias[:], in0=b_ap, in1=tt[:],
                                    op=AL.subtract)

        scale1 = sm([128, 1], "scale1")
        bias1 = sm([128, 1], "bias1")
        scale2 = sm([128, 1], "scale2")
        bias2 = sm([128, 1], "bias2")
        scale3 = sm([128, 2], "scale3")
        bias3 = sm([128, 2], "bias3")
        scmat1 = sm([128, SLAB], "scmat1")
        bimat1 = sm([128, SLAB], "bimat1")

        with tc.tile_pool(name="y1p", bufs=1) as y1pool:
            y1 = y1pool.tile([128, Y1COLS], c2dt, tag="y1", name="y1")
            # zero in half-slice chunks, low-d halves first: the dp-major
            # BN1 apply touches the low-d half of every slice first, so
            # those chunks must clear the (serial) Pool engine early
            for h in range(2):
                for s in range(SLAB):
                    lo = 1 + s * PSL + h * (PSL // 2)
                    hi = lo + PSL // 2
                    if h == 0 and s == 0:
                        lo = 0
                    if h == 1 and s == SLAB - 1:
                        hi = Y1COLS
                    nc.gpsimd.memset(y1[:, lo:hi], 0.0)

            y1lo5 = y1[0:64, 1:1 + SLAB * PSL].rearrange(
                "p (s d h w) -> p s d h w", s=SLAB, d=PD, h=PH, w=PW)
            y1all5 = y1[:, 1:1 + SLAB * PSL].rearrange(
                "p (s d h w) -> p s d h w", s=SLAB, d=PD, h=PH, w=PW)
            y1up5 = y1[64:128, 1:1 + SLAB * PSL].rearrange(
                "p (s d h w) -> p s d h w", s=SLAB, d=PD, h=PH, w=PW)

            # ======== conv1 (single pass) =============================
            # Owned slices (stats) first so the BN1 AllReduce fires early;
            # halo slices + staging overlap its latency. psum halves are
            # identical (w1's M-dup), giving free sources for the two
            # (lower / shifted-upper) y1 writes later.
            y1raw = y1pool.tile([128, 2 * NSP_OWN * 512], c2dt,
                                tag="y1raw", name="y1raw")
            s_order = (1, 2, 0, 3)
            uidx = {(s, dp): i * 8 + dp for i, s in enumerate(s_order)
                    for dp in range(8)}
            # owned first (stats -> AR1 fires early); halo dp-major so the
            # dp-major BN1 apply can start on dp=0 as soon as possible
            items = ([(1, dp) for dp in range(8)] + [(2, dp) for dp in range(8)]
                     + [(s, dp) for dp in range(4) for s in (0, 3)]
                     + [(s, dp) for dp in range(4, 8) for s in (0, 3)])
            with tc.tile_pool(name="ps2", bufs=8, space="PSUM") as ps2:
                for s, dp in items:
                    owned = s in (1, 2)
                    if True:
                        ps = ps2.tile([128, 512], f32, tag="c1p2")
                        for cb in range(2):
                            nc.tensor.matmul(
                                ps[:],
                                lhsT=w1sb[:, cb * 2 * PL:(cb + 1) * 2 * PL],
                                rhs=xslice(s, cb, dp),
                                start=(cb == 0), stop=(cb == 1),
                            )
                        u = uidx[(s, dp)]
                        if owned:
                            sp = (s - 1) * 8 + dp
                            nc.vector.bn_stats(
                                out=st1[:, sp * 6:(sp + 1) * 6],
                                in_=ps[0:64, :])
                        # drain psum: stats on DVE, owned copies on ACT
                        # (so ACT reaches the apply burst early), halo
                        # copies on DVE pushed late in the scheduler's
                        # virtual time so they don't park ahead of the
                        # bn_aggr -> AR1 chain
                        if owned:
                            nc.scalar.copy(
                                out=y1raw[:, u * 512:(u + 1) * 512],
                                in_=ps[:])
                        else:
                            with tc.tile_wait_until(0.5):
                                nc.vector.tensor_copy(
                                    out=y1raw[:, u * 512:(u + 1) * 512],
                                    in_=ps[:])
                        if s == 2 and dp == 7:
                            # owned stats complete: start the AR1 chain now.
                            # Its small DMAs ride the ACT queue (the SP
                            # queue is head-of-line blocked by halo-x
                            # ring-buffer waits) and high_priority jumps
                            # them ahead of the staging-copy backlog.
                            with tc.high_priority():
                                mv1 = sm([64, 2], "mv1")
                                arin1 = sm([64, 2], "arin1")
                                nc.vector.bn_aggr(out=mv1[:], in_=st1[:])
                                bn_reduce_prep(mv1, arin1)
                                nc.sync.dma_start(out=cc1_in[:],
                                                  in_=arin1[:])
                                allreduce(cc1_in[:], cc1_out[:])
                                g1s = sm([128, 2], "g1s")
                                nc.sync.dma_start(out=g1s[0:64, :],
                                                  in_=cc1_out[:])
                                nc.sync.dma_start(out=g1s[64:128, :],
                                                  in_=cc1_out[:])
                                bn_finalize(g1s, 1.0 / NCORES, gb1sb[:, 0:1],
                                            gb1sb[:, 1:2], scale1, bias1,
                                            "bn1")
                                # per-slice masked scale/bias (zero invalid)
                                nc.vector.tensor_scalar_mul(scmat1[:],
                                                            tmsb[:],
                                                            scale1[:])
                                nc.vector.tensor_scalar_mul(bimat1[:],
                                                            tmsb[:],
                                                            bias1[:])

            # ---- BN1 + relu into the padded y1 layout ----------------
            # dp-major so conv2's dp=0 can start as early as possible.
            # Lower half: unshifted; upper half: shifted one w-column left
            # (upper[w] = y1[w+1]) to enable conv2's kw-pair K-packing.
            # tile_wait_until keeps the scheduler from parking these
            # scale1-gated ops at the ACT queue head in front of the
            # (much earlier ready) psum staging copies.
            st.enter_context(tc.tile_wait_until(1))
            # merged over dp-pairs: y1raw blocks for (s, 2q) and (s, 2q+1)
            # are adjacent, and so are their d-windows in y1
            for q in range(4):
                dlo = 4 * q + 1
                for s in range(SLAB):
                    u = uidx[(s, 2 * q)]
                    rlo = y1raw[0:64, u * 512:(u + 2) * 512].rearrange(
                        "p (d h w) -> p d h w", d=4, h=16, w=16)
                    rhi = y1raw[64:128, u * 512:(u + 2) * 512].rearrange(
                        "p (d h w) -> p d h w", d=4, h=16, w=16)
                    nc.scalar.activation(
                        y1lo5[:, s, dlo:dlo + 4, 1:17, 1:17], rlo, AF.Relu,
                        bias=bimat1[0:64, s:s + 1], scale=scmat1[0:64, s:s + 1])
                    nc.scalar.activation(
                        y1up5[:, s, dlo:dlo + 4, 1:17, 0:16], rhi, AF.Relu,
                        bias=bimat1[64:128, s:s + 1],
                        scale=scmat1[64:128, s:s + 1])

            # ======== conv2: 27 spatial taps x 4 slices, packed =======
            # Per d-pair dp one PSUM bank accumulates both owned t-outs:
            # psum[0:64] = t-out s=1, psum[64:128] = t-out s=2.
            def win(view, s, dp, kd, kh, kw):
                d0 = 2 * dp + kd
                return view[:, s, d0:d0 + 2, kh:kh + 16, kw:kw + 16]

            with tc.tile_pool(name="ps3", bufs=4, space="PSUM") as ps3:
                for dp in range(8):
                    ps = ps3.tile([128, 512], f32, tag="c2p")
                    first = [True]

                    def mm(outap, lhsT, rhs, stop=False):
                        nc.tensor.matmul(outap, lhsT=lhsT, rhs=rhs,
                                         start=first[0], stop=stop)
                        first[0] = False

                    for kd in range(3):
                        for kh in range(3):
                            b = (kd * 3 + kh)
                            # s'=1: M=128 pairs (kw 0+1) and kw=2 singles
                            mm(ps[:], w2sb[:, W2_PAIR1 + b * 128:
                                           W2_PAIR1 + (b + 1) * 128],
                               win(y1all5, 1, dp, kd, kh, 0))
                            mm(ps[:], w2sb[0:64, W2_SING12 + b * 128:
                                           W2_SING12 + (b + 1) * 128],
                               win(y1lo5, 1, dp, kd, kh, 2))
                            # s'=2: M=128
                            mm(ps[:], w2sb[:, W2_PAIR2 + b * 128:
                                           W2_PAIR2 + (b + 1) * 128],
                               win(y1all5, 2, dp, kd, kh, 0))
                            mm(ps[:], w2sb[64:128, W2_SING12 + b * 128:
                                           W2_SING12 + (b + 1) * 128],
                               win(y1up5, 2, dp, kd, kh, 1))
                    for kd in range(3):
                        for kh in range(3):
                            b = (kd * 3 + kh)
                            last = (kd == 2 and kh == 2)
                            # s'=0 -> only t-out s=1 (psum[0:64])
                            mm(ps[0:64, :], w2sb[:, W2_PAIR0 + b * 64:
                                                 W2_PAIR0 + (b + 1) * 64],
                               win(y1all5, 0, dp, kd, kh, 0))
                            mm(ps[0:64, :], w2sb[0:64, W2_SING03 + b * 64:
                                                 W2_SING03 + (b + 1) * 64],
                               win(y1lo5, 0, dp, kd, kh, 2), stop=last)
                    for kd in range(3):
                        for kh in range(3):
                            b = (kd * 3 + kh)
                            last = (kd == 2 and kh == 2)
                            # s'=3 -> only t-out s=2 (psum[64:128])
                            mm(ps[64:128, :], w2sb[:, W2_PAIR3 + b * 64:
                                                   W2_PAIR3 + (b + 1) * 64],
                               win(y1all5, 3, dp, kd, kh, 0))
                            mm(ps[64:128, :], w2sb[64:128, W2_SING03 + b * 64:
                                                   W2_SING03 + (b + 1) * 64],
                               win(y1up5, 3, dp, kd, kh, 1), stop=last)

                    nc.scalar.copy(out=y2[:, dp * 512:(dp + 1) * 512],
                                   in_=ps[:])
                    nc.vector.bn_stats(out=st2[:, dp * 6:(dp + 1) * 6],
                                       in_=y2[:, dp * 512:(dp + 1) * 512])

        # y1 freed here
        with tc.high_priority():
            mv2 = sm([128, 2], "mv2")
            arin2 = sm([128, 2], "arin2")
            nc.vector.bn_aggr(out=mv2[:], in_=st2[:])
            bn_reduce_prep(mv2, arin2)
            nc.sync.dma_start(out=cc2_in[:], in_=arin2[:])
            allreduce(cc2_in[:], cc2_out[:])
            fa = sm([128, 2], "fa2")
            fb = sm([128, 2], "fb2")
            nc.sync.dma_start(out=fa[:], in_=cc2_out[:])
            nc.sync.dma_start(out=fb[0:64, :], in_=cc2_out[64:128, :])
            nc.sync.dma_start(out=fb[64:128, :], in_=cc2_out[0:64, :])
            nc.vector.tensor_tensor(out=fa[:], in0=fa[:], in1=fb[:],
                                    op=AL.add)
            bn_finalize(fa, 1.0 / (2 * NCORES), gb2sb[:, 0:1], gb2sb[:, 1:2],
                        scale2, bias2, "bn2")

        # residual x tiles: 4 x [128, 4096] fp16; tile j = oh*2 + h covers
        # positions [h*4096, (h+1)*4096) of output-channel half oh. The
        # pool opens after y1 is freed, reusing its SBUF zone.
        x9p = st.enter_context(tc.tile_pool(name="x9p", bufs=4))
        xt9 = [x9p.tile([128, 4096], fp16, tag="xt9", name=f"xt9_{j}")
               for j in range(4)]
        # residual prefetch on the (idle by now) Pool queue: the dummy
        # scale2-dependent write pins these DMAs to the DMA-idle BN2
        # window, so their transfers can't steal bandwidth from the conv1
        # x stream (the tile scheduler is readiness-driven)
        for j in range(4):
            nc.vector.tensor_copy(out=xt9[j][:, 0:1], in_=scale2[:])
            nc.gpsimd.dma_start(
                out=xt9[j][:],
                in_=xres[j // 2, :, (j % 2) * 4096:(j % 2 + 1) * 4096])

        with tc.tile_pool(name="zp", bufs=1) as zpool:
            y2n = zpool.tile([128, NSP_OWN * 256], mmdt, tag="y2n", name="y2n")

            def c3_mm(ps4, dp, half, oh, stop=True):
                rhs = y2n[half * 64:(half + 1) * 64,
                          dp * 512:(dp + 1) * 512].bitcast(mmdt)
                pg = ps4.tile([128, 512], f32, tag="c3")
                nc.tensor.matmul(
                    pg[:],
                    lhsT=w3sb[half * 64:(half + 1) * 64,
                              oh * 128:(oh + 1) * 128].bitcast(mmdt),
                    rhs=rhs, start=True, stop=stop)
                return pg

            # ==== BN2 apply + conv3 pass 1 (stats), interleaved =======
            # stats alternate between DVE-direct-from-psum and an
            # ACT-copy-to-bf16 + DVE 2x-rate pass, halving the DVE
            # serialization that otherwise paces this phase
            with tc.tile_pool(name="ps4", bufs=8, space="PSUM") as ps4, \
                 tc.tile_pool(name="y3st", bufs=8) as y3stp:
                for dp in range(8):
                    nc.scalar.activation(
                        y2n[:, dp * 512:(dp + 1) * 512],
                        y2[:, dp * 512:(dp + 1) * 512],
                        AF.Relu, bias=bias2[:], scale=scale2[:])
                    for half in range(2):
                        sp = half * 8 + dp
                        for oh in range(2):
                            pg = c3_mm(ps4, dp, half, oh)
                            stout = st3[:, (oh * 16 + sp) * 6:
                                        (oh * 16 + sp) * 6 + 6]
                            if half == 0:
                                nc.vector.bn_stats(out=stout, in_=pg[:])
                            else:
                                yst = y3stp.tile([128, 512], c2dt,
                                                 tag="y3st")
                                nc.scalar.copy(out=yst[:], in_=pg[:])
                                nc.vector.bn_stats(out=stout, in_=yst[:])

            # BN3 stats merge, vectorized [128,2]-wide over both oh halves:
            # arin3 cols = (mean0, mean1, e2_0, e2_1)
            with tc.high_priority():
                mv3a = sm([128, 2], "mv3a")
                mv3b = sm([128, 2], "mv3b")
                arin3 = sm([128, 4], "arin3")
                sq3 = sm([128, 2], "sq3")
                nc.vector.bn_aggr(out=mv3a[:], in_=st3[:, 0:96])
                nc.vector.bn_aggr(out=mv3b[:], in_=st3[:, 96:192])
                nc.vector.tensor_copy(out=arin3[:, 0:1], in_=mv3a[:, 0:1])
                nc.vector.tensor_copy(out=arin3[:, 1:2], in_=mv3b[:, 0:1])
                nc.vector.tensor_copy(out=arin3[:, 2:3], in_=mv3a[:, 1:2])
                nc.vector.tensor_copy(out=arin3[:, 3:4], in_=mv3b[:, 1:2])
                nc.vector.tensor_tensor(out=sq3[:], in0=arin3[:, 0:2],
                                        in1=arin3[:, 0:2], op=AL.mult)
                nc.vector.tensor_tensor(out=arin3[:, 2:4], in0=arin3[:, 2:4],
                                        in1=sq3[:], op=AL.add)
                nc.sync.dma_start(out=cc3_in[:], in_=arin3[:])
                allreduce(cc3_in[:], cc3_out[:])
                g3s = sm([128, 4], "g3s")
                nc.sync.dma_start(out=g3s[:], in_=cc3_out[:])
                recip3 = sm([128, 2], "recip3")
                me3 = sm([128, 4], "me3")
                v3 = sm([128, 2], "v3")
                rstd3 = sm([128, 2], "rstd3")
                nc.vector.tensor_scalar_mul(me3[:], g3s[:], 1.0 / NCORES)
                nc.vector.tensor_tensor(out=sq3[:], in0=me3[:, 0:2],
                                        in1=me3[:, 0:2], op=AL.mult)
                nc.vector.tensor_tensor(out=v3[:], in0=me3[:, 2:4],
                                        in1=sq3[:], op=AL.subtract)
                nc.vector.tensor_scalar_add(v3[:], v3[:], EPS)
                nc.vector.reciprocal(out=v3[:], in_=v3[:])
                nc.scalar.activation(rstd3[:], v3[:], AF.Sqrt)
                nc.vector.tensor_tensor(out=scale3[:], in0=gb3sb[:, 0:2],
                                        in1=rstd3[:], op=AL.mult)
                nc.vector.tensor_tensor(out=sq3[:], in0=me3[:, 0:2],
                                        in1=scale3[:], op=AL.mult)
                nc.vector.tensor_tensor(out=bias3[:], in0=gb3sb[:, 2:4],
                                        in1=sq3[:], op=AL.subtract)
                nc.vector.reciprocal(out=recip3[:], in_=scale3[:])
                for oh in range(2):
                    # diag(1/scale3): PE injects the residual into PSUM
                    nc.vector.tensor_scalar_mul(
                        diag3[:, oh * 128:(oh + 1) * 128], idmsb[:],
                        recip3[:, oh:oh + 1])

            # ==== conv3 pass 2 + fused BN3/residual/relu/store ========
            # Stores batched as [128,1024] per (oh, half, dp-pair).
            with tc.tile_pool(name="ps5", bufs=8, space="PSUM") as ps5, \
                 tc.tile_pool(name="fino", bufs=8) as fino:
                ftiles = {}
                for dp in range(8):
                    for half in range(2):
                        sp = half * 8 + dp
                        for oh in range(2):
                            pg = c3_mm(ps5, dp, half, oh, stop=False)
                            nc.tensor.matmul(
                                pg[:], lhsT=diag3[:, oh * 128:(oh + 1) * 128],
                                rhs=xt9[oh * 2 + half][:, dp * 512:
                                                       (dp + 1) * 512],
                                start=False, stop=True)
                            if dp % 2 == 0:
                                ftiles[(oh, half)] = fino.tile(
                                    [128, 1024], c2dt, tag="o9",
                                    name=f"o9_{dp}_{half}_{oh}")
                            ft = ftiles[(oh, half)]
                            slot = ft[:, (dp % 2) * 512:(dp % 2) * 512 + 512]
                            if dp % 2 == 1 and half == 1:
                                nc.vector.tensor_scalar(
                                    slot, pg[:], scale3[:, oh:oh + 1],
                                    bias3[:, oh:oh + 1],
                                    op0=AL.mult, op1=AL.add)
                                nc.vector.tensor_scalar_max(slot, slot, 0.0)
                            else:
                                nc.scalar.activation(
                                    slot, pg[:], AF.Relu,
                                    bias=bias3[:, oh:oh + 1],
                                    scale=scale3[:, oh:oh + 1])
                            if dp % 2 == 1:
                                nc.sync.dma_start(
                                    out=out[oh, :, sp * 512 - 512:
                                            sp * 512 + 512],
                                    in_=ft[:])

    nc.compile()
    return nc


# ---- host-side input prep / output assembly -----------------------------

def _prep_w2(w2):
    """Pack w2 [64,64,3,3,3,3] into the [128, W2_COLS] lhsT tile layout.

    kt for (t-out column half m-half, input slice s') is kt = s' - s_out + 1
    with s_out = 1 + m_half. Pair blocks: rows 0:64 = tap kw (=0), rows
    64:128 = tap kw+1 (=1, via the shifted y1 copy). Singles: tap kw=2.
    """
    f4 = np.float32
    w = np.ascontiguousarray(w2, f4)  # [o, c, kt, kd, kh, kw]
    res = np.zeros((128, W2_COLS), f4)
    for kd in range(3):
        for kh in range(3):
            b = kd * 3 + kh
            # pairs for s'=1, s'=2 (M=128)
            for base, sp in ((W2_PAIR1, 1), (W2_PAIR2, 2)):
                for mh in range(2):     # m-half: t-out s_out = 1 + mh
                    kt = sp - (1 + mh) + 1
                    for kw in range(2):
                        res[kw * 64:kw * 64 + 64,
                            base + b * 128 + mh * 64:
                            base + b * 128 + mh * 64 + 64] = \
                            w[:, :, kt, kd, kh, kw].T
            # pairs for s'=0 (t-out 1, kt=0) / s'=3 (t-out 2, kt=2)
            for base, kt in ((W2_PAIR0, 0), (W2_PAIR3, 2)):
                for kw in range(2):
                    res[kw * 64:kw * 64 + 64,
                        base + b * 64:base + b * 64 + 64] = \
                        w[:, :, kt, kd, kh, kw].T
            # kw=2 singles: s'=1 rows 0:64, s'=2 rows 64:128 (M=128)
            for rh, sp in ((0, 1), (1, 2)):
                for mh in range(2):
                    kt = sp - (1 + mh) + 1
                    res[rh * 64:rh * 64 + 64,
                        W2_SING12 + b * 128 + mh * 64:
                        W2_SING12 + b * 128 + mh * 64 + 64] = \
                        w[:, :, kt, kd, kh, 2].T
            # kw=2 singles: s'=0 rows 0:64 (kt=0), s'=3 rows 64:128 (kt=2)
            for rh, kt in ((0, 0), (1, 2)):
                res[rh * 64:rh * 64 + 64,
                    W2_SING03 + b * 64:W2_SING03 + b * 64 + 64] = \
                    w[:, :, kt, kd, kh, 2].T
    return res


def _prep_inputs(x, w1, g1, b1, w2, g2, b2, w3, g3, b3):
    import ml_dtypes
    f4 = np.float32
    bf = ml_dtypes.bfloat16
    xr = np.ascontiguousarray(x, f4).reshape(B, C, T, DHW)

    w2t = _prep_w2(w2).astype(bf)

    w1T = np.ascontiguousarray(w1, f4).T.reshape(2, 128, PL)  # [cb, k, o]
    w1t = np.ascontiguousarray(
        np.concatenate([np.concatenate([w1T[cb]] * 2, 1) for cb in range(2)], 1)
    ).astype(bf)
    w3t = np.concatenate([np.ascontiguousarray(w3, f4).T] * 2, 0).copy()

    gb1 = np.stack([np.asarray(g1, f4), np.asarray(b1, f4)], 1)
    gb1 = np.concatenate([gb1, gb1], 0)
    gb2 = np.stack([np.asarray(g2, f4), np.asarray(b2, f4)], 1)
    gb2 = np.concatenate([gb2, gb2], 0)
    g3r = np.asarray(g3, f4).reshape(2, 128).T
    b3r = np.asarray(b3, f4).reshape(2, 128).T
    gb3 = np.concatenate([g3r, b3r], 1).copy()  # [128,4]

    in_maps = []
    for core in range(NCORES):
        b = core // 4
        t0 = 2 * (core % 4)
        xslab = np.zeros((C, SLAB, DHW), f4)
        tm = np.zeros((SLAB,), f4)
        for si, gt in enumerate(range(t0 - 1, t0 + 3)):
            if 0 <= gt < T:
                xslab[:, si] = xr[b, :, gt]
                tm[si] = 1.0
        xs2 = xslab.reshape(2, 128, SLAB * DHW)
        in_maps.append({
            "xsb": np.ascontiguousarray(xs2).astype(bf),
            "xres": np.ascontiguousarray(
                xs2[:, :, DHW:DHW + NPOS_OWN]).astype(np.float16),
            "idm": np.eye(128, dtype=np.float16),
            "w1t": w1t, "w2t": w2t, "w3t": w3t,
            "gb1": gb1, "gb2": gb2, "gb3": gb3,
            "tmask": np.broadcast_to(tm, (128, SLAB)).copy(),
        })
    return in_maps


def kernel(x, w1, g1, b1, w2, g2, b2, w3, g3, b3):
    global LAST_RESULT
    from concourse.bass_utils import run_bass_kernel_spmd

    nc = _build(MM_DT, C2_DT)
    in_maps = _prep_inputs(x, w1, g1, b1, w2, g2, b2, w3, g3, b3)
    res = run_bass_kernel_spmd(nc, in_maps, core_ids=list(range(NCORES)))
    LAST_RESULT = res

    full = np.empty((B, C, T, D, H, W), np.float32)
    for core in range(NCORES):
        b = core // 4
        t0 = 2 * (core % 4)
        o = np.asarray(res.results[core]["out"], np.float32)
        full[b, :, t0:t0 + TPC] = o.reshape(C, TPC, D, H, W)
    return full
